# revision 54
# baseline (speedup 1.0000x reference)
"""Self-contained Trainium2 Bass kernel for nn_Attention_xxc_52390011077379.

kernel(**inputs) takes FULL inputs:
  x [8192, 17, 512] f32, W_qkv [512, 1536], W_proj [512, 512], b_proj [512]
returns FULL output [8192, 17, 512] f32.

Strategy: pure data parallelism over the batch axis across 8 NeuronCores
(1024 batches/core, padded to 1036 = 148 groups of 7 for the on-chip
grouped-attention pipeline); weights replicated.
"""
import numpy as np
import concourse.bacc as bacc
import concourse.mybir as mybir
from concourse.tile import TileContext

FP32 = mybir.dt.float32
F32R = mybir.dt.float32r
F16 = mybir.dt.float16
AF = mybir.ActivationFunctionType
ALU = mybir.AluOpType

BONECHAIN = [[0, 1, 2, 3], [0, 4, 5, 6], [0, 7, 8, 9, 10], [8, 11, 12, 13], [8, 14, 15, 16]]
CHAIN_STEPS = [(c[i - 1], c[i], c[i + 1]) for c in BONECHAIN for i in range(1, len(c) - 1)]

N = 17
C = 512
H = 8
HD = 64
SCALE = HD ** -0.5
GB = 7
GR = GB * N  # 119

N_CORES = 8
B_FULL = 8192
B_CORE = B_FULL // N_CORES     # 1024
B_PAD = 1036                   # 148 groups of 7
G_CHUNK = 10
CONV1A_ENG = 'sync'
CONV2B_ENG = 'scalar'
BUFS_BIG = 2
BUFS_BM = 1
BUFS_SS = 2
BUFS_QKT = 1
BUFS_V = 3
BUFS_OUT = 1

_CACHE = {}
TIMING_REPEAT = 5


def _build(nc, B_pad, G_chunk=16, mm_dt=F32R, att_dt=F16, repeat=1):
    assert B_pad % GB == 0
    n_groups = B_pad // GB
    chunks = []
    g0 = 0
    while g0 < n_groups:
        g = min(G_chunk, n_groups - g0)
        chunks.append((g0, g))
        g0 += g

    R_tot = B_pad * N

    x_d = nc.dram_tensor("x", [R_tot, C], FP32, kind="ExternalInput")
    wqkv_d = nc.dram_tensor("w_qkv", [C, 3 * C], FP32, kind="ExternalInput")
    wproj_d = nc.dram_tensor("w_proj", [C, C], FP32, kind="ExternalInput")
    bproj_d = nc.dram_tensor("b_proj", [1, C], FP32, kind="ExternalInput")
    y_d = nc.dram_tensor("y", [R_tot, C], FP32, kind="ExternalOutput")

    ident_d = nc.inline_tensor(np.eye(128, dtype=np.float32), name="ident128")

    with TileContext(nc) as tc:
        with tc.tile_pool(name="persist", bufs=1) as pp, \
             tc.tile_pool(name="xin", bufs=1) as xin_p, \
             tc.tile_pool(name="big", bufs=BUFS_BIG) as big_p, \
             tc.tile_pool(name="qkT", bufs=BUFS_QKT) as qkT_p, \
             tc.tile_pool(name="vp", bufs=BUFS_V) as v_p, \
             tc.tile_pool(name="sstrip", bufs=BUFS_SS) as ss_p, \
             tc.tile_pool(name="bmaj", bufs=BUFS_BM) as bm_p, \
             tc.tile_pool(name="outp", bufs=BUFS_OUT) as out_p, \
             tc.tile_pool(name="dram", bufs=2, space="DRAM") as dram_p, \
             tc.tile_pool(name="ps", bufs=8, space="PSUM") as ps_p:

            ident = pp.tile([128, 128], FP32)
            nc.sync.dma_start(out=ident[:], in_=ident_d[:])

            bias_b = pp.tile([128, C], FP32)
            btmp = pp.tile([1, C], FP32)
            nc.sync.dma_start(out=btmp[:], in_=bproj_d[:])
            nc.gpsimd.partition_broadcast(bias_b[:], btmp[:])

            wqkv_r = []
            wpj_r = []
            for kt in range(4):
                wt = xin_p.tile([128, 3 * C], FP32, tag="wtmp", name="wt")
                nc.scalar.dma_start(out=wt[:], in_=wqkv_d[kt * 128:(kt + 1) * 128, :])
                wr = pp.tile([128, 3 * C], att_dt, tag=f"wqkv_{kt}", name="wr")
                nc.vector.tensor_copy(wr[:], wt[:])
                wqkv_r.append(wr)
                wt2 = xin_p.tile([128, C], FP32, tag="wtmp2", name="wt2")
                nc.scalar.dma_start(out=wt2[:], in_=wproj_d[kt * 128:(kt + 1) * 128, :])
                wr2 = pp.tile([128, C], att_dt, tag=f"wpj_{kt}", name="wr2")
                nc.vector.tensor_copy(wr2[:], wt2[:])
                wpj_r.append(wr2)

            def wqk(mt, kt):
                return wqkv_r[kt][:, mt * 128:(mt + 1) * 128]

            def wv(kt):
                return wqkv_r[kt][:, 1024:1536]

            def wpj(kt):
                return wpj_r[kt][:]

            # Persistent AT strip panels (x2, alternating by chunk parity),
            # (h, g)-major blocks of GR cols.
            at_strips = []
            for pi in range(3):
                at_s = pp.tile([GR, H * G_chunk * GR], att_dt,
                               tag=f"atstrip{pi}", name=f"at_s{pi}")
                nc.vector.memset(at_s[:], 0.0)
                at_strips.append(at_s)

            def emit_front(ci):
                g0, G = chunks[ci]
                RC = GR * G
                r0 = g0 * GR

                # A1: load x in half-chunks, PE-transpose to xT
                n_rt = (RC + 127) // 128
                xT = big_p.tile([128, 4 * RC], att_dt, tag="bigbuf", name="xT")
                HALF_RT = 5
                for h0 in range(0, n_rt, HALF_RT):
                    h_rt = min(HALF_RT, n_rt - h0)
                    rows0 = h0 * 128
                    rows = min(RC - rows0, h_rt * 128)
                    full_rt = rows // 128
                    rem = rows - full_rt * 128
                    xin_t = xin_p.tile([128, HALF_RT * C], FP32, tag="xin", name="xin_t")
                    if full_rt:
                        nc.sync.dma_start(
                            out=xin_t[:, :full_rt * C].rearrange(
                                "p (rt c) -> p rt c", rt=full_rt),
                            in_=x_d[r0 + rows0: r0 + rows0 + full_rt * 128, :].rearrange(
                                "(rt p) c -> p rt c", p=128))
                    if rem:
                        nc.sync.dma_start(
                            out=xin_t[:rem, full_rt * C: (full_rt + 1) * C],
                            in_=x_d[r0 + rows0 + full_rt * 128: r0 + rows0 + rows, :])
                    for rt in range(h_rt):
                        rr0 = rows0 + rt * 128
                        rr = min(128, RC - rr0)
                        pst = ps_p.tile([128, 512], FP32, tag="ps", name="pst")
                        for k in range(4):
                            nc.tensor.transpose(
                                pst[:, k * 128:k * 128 + rr],
                                xin_t[:rr, rt * C + k * 128: rt * C + (k + 1) * 128],
                                ident[:rr, :rr])
                        dst = xT[:].rearrange("p (k r) -> p k r", k=4)[:, :, rr0:rr0 + rr]
                        srcc = pst[:].rearrange("p (k r) -> p k r", k=4)[:, :, :rr]
                        nc.vector.tensor_copy(dst, srcc)

                def xTk(k):
                    return xT[:, k * RC:(k + 1) * RC]

                # A2: qT,kT (ch-major f16); copies split DVE/ACT
                qkT = [qkT_p.tile([128, RC], att_dt, tag=f"qkT{mt}", name=f"qkT{mt}")
                       for mt in range(8)]
                n_nt = (RC + 475) // 476
                for mt in range(8):
                    for nt in range(n_nt):
                        c0 = nt * 476
                        cw = min(476, RC - c0)
                        psq = ps_p.tile([128, 512], FP32, tag="ps", name="psq")
                        for kt in range(4):
                            nc.tensor.matmul(
                                psq[:, :cw], wqk(mt, kt),
                                xTk(kt)[:, c0:c0 + cw],
                                start=(kt == 0), stop=(kt == 3))
                        if mt % 2 == 0:
                            nc.vector.tensor_copy(qkT[mt][:, c0:c0 + cw], psq[:, :cw])
                        else:
                            nc.scalar.copy(qkT[mt][:, c0:c0 + cw], psq[:, :cw])

                # A3: v (row-major f16) per group
                vts = []
                for g in range(G):
                    vt = v_p.tile([GR, C], att_dt, tag=f"v{g}", name=f"v{g}")
                    psv = ps_p.tile([128, 512], FP32, tag="ps", name="psv")
                    for kt in range(4):
                        nc.tensor.matmul(
                            psv[:GR, :], xTk(kt)[:, g * GR:(g + 1) * GR], wv(kt),
                            start=(kt == 0), stop=(kt == 3))
                    nc.vector.tensor_copy(vt[:], psv[:GR, :])
                    vts.append(vt)

                # B + conv1a: scores by gh-halves -> staged S in DRAM
                # staged S layout [j:7][n:17][g:G][h:8][m:17] f16
                stS = dram_p.tile([7, N * G * H * N], att_dt, tag="stagedS")
                n_gh = G * H
                GH_HALF = 16
                for gh0 in range(0, n_gh, GH_HALF):
                    ghw = min(GH_HALF, n_gh - gh0)
                    g_lo = gh0 // H
                    g_hi = (gh0 + ghw) // H
                    sstrip = ss_p.tile([GR, GH_HALF * GR], att_dt, tag="ss", name="sstrip")
                    for g in range(g_lo, g_hi):
                        for par in range(2):
                            pss = ps_p.tile([128, 512], FP32, tag="ps", name="pss")
                            for qi in range(4):
                                h = 2 * qi + par
                                mt = h // 2
                                p0 = par * 64
                                qs = qkT[mt][p0:p0 + 64, g * GR:(g + 1) * GR]
                                ks = qkT[4 + mt][p0:p0 + 64, g * GR:(g + 1) * GR]
                                nc.tensor.matmul(pss[:GR, qi * GR:(qi + 1) * GR],
                                                 qs, ks, start=True, stop=True)
                            bidx = (g - g_lo) * H + par
                            dst = sstrip[:].rearrange(
                                "p (hh q) -> p hh q", q=GR)[:, bidx:bidx + 7:2, :]
                            srcq = pss[:GR, :4 * GR].rearrange(
                                "p (hh q) -> p hh q", q=GR)
                            if par == 0:
                                nc.vector.tensor_copy(dst, srcq)
                            else:
                                nc.scalar.copy(dst, srcq)
                    for j in range(7):
                        srcj = sstrip[N * j:N * (j + 1), :ghw * GR] \
                            .rearrange("p (gh m) -> p gh m", m=GR)[:, :, N * j:N * (j + 1)]
                        dstj = stS[j:j + 1, :] \
                            .rearrange("o (n gh m) -> (o n) gh m", n=N, m=N)[:, gh0:gh0 + ghw, :]
                        getattr(nc, CONV1A_ENG).dma_start(out=dstj, in_=srcj)
                return {"vts": vts, "stS": stS, "G": G, "RC": RC, "r0": r0, "ci": ci}

            def emit_spine(st):
                stS = st["stS"]
                G, RC, r0 = st["G"], st["RC"], st["r0"]
                BC = GB * G
                at_strip = at_strips[st["ci"] % 3]

                # conv1b: staged -> b-major
                bmS = bm_p.tile([BC, N * H * N], att_dt, tag="bmS")
                for j in range(7):
                    srcc = stS[j:j + 1, :].rearrange(
                        "o (n g h m) -> (o g) n (h m)", n=N, g=G, h=H)
                    dst = bmS[j * G:(j + 1) * G, :].rearrange(
                        "b (n hm) -> b n hm", n=N)
                    nc.scalar.dma_start(out=dst, in_=srcc)

                # D: chain + softmax in b-major
                bm4 = bmS[:].rearrange("b (n h m) -> b n h m", n=N, h=H)
                for (pp_, p_, c_) in CHAIN_STEPS:
                    nc.vector.tensor_tensor(
                        out=bm4[:, p_, :, c_], in0=bm4[:, p_, :, c_],
                        in1=bm4[:, pp_, :, p_], op=ALU.add)
                    nc.vector.tensor_tensor(
                        out=bm4[:, c_, :, p_], in0=bm4[:, c_, :, p_],
                        in1=bm4[:, pp_, :, p_], op=ALU.add)
                    nc.vector.tensor_scalar_mul(bm4[:, p_, :, c_], bm4[:, p_, :, c_], 0.5)
                    nc.vector.tensor_scalar_mul(bm4[:, c_, :, p_], bm4[:, c_, :, p_], 0.5)

                # per-h exp/softmax into bmA_h [(b), (m n)]; conv2 via
                # staged3 [m][(j g h)][n]: 8 floor-writes + 7 fused j-reads
                bmA_h = [bm_p.tile([BC, N * N], att_dt, tag=f"bmA{h}", name=f"bmA{h}")
                         for h in range(H)]
                zs = bm_p.tile([BC, N * H], FP32, tag="zs")
                rec = bm_p.tile([BC, N * H], FP32, tag="rec")
                for h in range(H):
                    a4h = bmA_h[h][:].rearrange("b (m n) -> b n m", m=N)
                    nc.scalar.activation(a4h, bm4[:, :, h, :], AF.Exp, scale=SCALE)
                    nc.vector.tensor_reduce(
                        zs[:, h * N:(h + 1) * N], a4h, mybir.AxisListType.X, ALU.add)
                    nc.vector.reciprocal(rec[:, h * N:(h + 1) * N],
                                         zs[:, h * N:(h + 1) * N])
                    r4b = rec[:, h * N:(h + 1) * N].unsqueeze(2).broadcast_to(
                        [BC, N, N])
                    nc.vector.tensor_tensor(out=a4h, in0=a4h, in1=r4b, op=ALU.mult)

                staged3 = dram_p.tile([1, N * GB * G * H * N], att_dt, tag="stagedA")
                w_engs = [nc.sync, nc.scalar, nc.sync]
                for h in range(H):
                    dsth = staged3[:].rearrange(
                        "o (m jg hn) -> (o jg) m hn", m=N, hn=H * N)[
                            :, :, h * N:(h + 1) * N]
                    srch = bmA_h[h][:].rearrange("b (m n) -> b m n", m=N)
                    w_engs[h % 3].dma_start(out=dsth, in_=srch)
                for j in range(GB):
                    srcp = staged3[:].rearrange(
                        "o (m jgh n) -> (o m) jgh n", m=N, n=N)[
                            :, j * H * G:(j + 1) * H * G, :]
                    dstp = at_strip[N * j:N * (j + 1), :].rearrange(
                        "p (gh q) -> p gh q", q=GR)[:, :G * H, N * j:N * (j + 1)]
                    w_engs[j % 3].dma_start(out=dstp, in_=srcp)


            def emit_finish(st):
                vts = st["vts"]
                G, RC, r0 = st["G"], st["RC"], st["r0"]
                at_strip = at_strips[st["ci"] % 3]

                # F: AV; pack 4 g per PSUM bank per head-pair t
                aoT = big_p.tile([128, 4 * RC], att_dt, tag="bigbuf", name="aoT")

                def aoTk(t):
                    return aoT[:, t * RC:(t + 1) * RC]

                for q0 in range(0, G, 4):
                    nq = min(4, G - q0)
                    for t in range(4):
                        psa = ps_p.tile([128, 512], FP32, tag="ps", name="psa")
                        for qi in range(nq):
                            g = q0 + qi
                            for hp in range(2):
                                h = 2 * t + hp
                                nc.tensor.matmul(
                                    psa[64 * hp:64 * (hp + 1), qi * GR:(qi + 1) * GR],
                                    vts[g][:, h * HD:(h + 1) * HD],
                                    at_strip[:, (g * H + h) * GR:(g * H + h + 1) * GR],
                                    start=True, stop=True)
                        dst = aoTk(t)[:, q0 * GR:(q0 + nq) * GR]
                        if t % 2 == 0:
                            nc.vector.tensor_copy(dst, psa[:, :nq * GR])
                        else:
                            nc.scalar.copy(dst, psa[:, :nq * GR])

                # G: proj + bias into half out panels; 1 DMA per half
                G_HALF = min(5, G)
                for gg0 in range(0, G, G_HALF):
                    gw = min(G_HALF, G - gg0)
                    outp = out_p.tile([GR, G_HALF * C], FP32, tag="out", name="outp")
                    for gi in range(gw):
                        g = gg0 + gi
                        psp2 = ps_p.tile([128, 512], FP32, tag="ps", name="psp2")
                        for kt in range(4):
                            nc.tensor.matmul(
                                psp2[:GR, :], aoTk(kt)[:, g * GR:(g + 1) * GR], wpj(kt),
                                start=(kt == 0), stop=(kt == 3))
                        nc.vector.tensor_tensor(
                            out=outp[:, gi * C:(gi + 1) * C], in0=psp2[:GR, :],
                            in1=bias_b[:GR, :], op=ALU.add)
                    nc.sync.dma_start(
                        out=y_d[r0 + gg0 * GR: r0 + (gg0 + gw) * GR, :].rearrange(
                            "(g p) c -> p g c", p=GR),
                        in_=outp[:, :gw * C].rearrange("p (g c) -> p g c", g=gw))

            # software-pipelined emission:
            #   spine(i) right after front(i); front(i+1); then finish(i)
            pending = []
            cnum = 0
            for rep in range(repeat):
                for ci in range(len(chunks)):
                    st = emit_front(ci)
                    st["ci"] = cnum
                    cnum += 1
                    emit_spine(st)
                    pending.append(st)
                    if len(pending) > 2:
                        emit_finish(pending.pop(0))
            for st in pending:
                emit_finish(st)
    return nc


def _get_nc(repeat=1):
    key = (B_PAD, G_CHUNK, repeat)
    if key not in _CACHE:
        nc = bacc.Bacc(
            "TRN2", target_bir_lowering=False, debug=False,
            enable_asserts=False, num_devices=N_CORES,
        )
        _build(nc, B_pad=B_PAD, G_chunk=G_CHUNK, repeat=repeat)
        nc.compile()
        _CACHE[key] = nc
    return _CACHE[key]


def make_in_maps(inputs):
    x = np.asarray(inputs["x"], dtype=np.float32)
    W_qkv = np.asarray(inputs["W_qkv"], dtype=np.float32)
    W_proj = np.asarray(inputs["W_proj"], dtype=np.float32)
    b_proj = np.asarray(inputs["b_proj"], dtype=np.float32)
    in_maps = []
    for c in range(N_CORES):
        xs = x[c * B_CORE:(c + 1) * B_CORE]
        pad = np.zeros((B_PAD - B_CORE, N, C), np.float32)
        xs = np.concatenate([xs, pad], axis=0).reshape(-1, C)
        in_maps.append({
            "x": xs,
            "w_qkv": W_qkv,
            "w_proj": W_proj,
            "b_proj": b_proj.reshape(1, C),
        })
    return in_maps


def kernel(x, W_qkv, W_proj, b_proj, _repeat=1):
    from concourse.bass_utils import run_bass_kernel_spmd

    x = np.asarray(x, dtype=np.float32)
    B, N_, C_ = x.shape
    assert (B, N_, C_) == (B_FULL, N, C)

    nc = _get_nc(_repeat)
    in_maps = make_in_maps({"x": x, "W_qkv": W_qkv, "W_proj": W_proj,
                            "b_proj": b_proj})
    res = run_bass_kernel_spmd(nc, in_maps, list(range(N_CORES)))
    outs = []
    for c in range(N_CORES):
        yc = res.results[c]["y"].reshape(B_PAD, N, C)[:B_CORE]
        outs.append(yc)
    return np.concatenate(outs, axis=0)


def kernel_repeat(inputs, repeat):
    return kernel(**inputs, _repeat=repeat)
